# revision 13
# baseline (speedup 1.0000x reference)
"""MoE combiner kernel for Trainium2 (8 NeuronCores, SPMD).

Computes out[i, d] = sum_e gates[i, e] * expert_outputs[e, d]
  gates:          [16384, 64]  fp32 (top-2 sparse rows, but dense contraction
                                     moves less HBM traffic than a gather)
  expert_outputs: [64, 4096]   fp32
  out:            [16384, 4096] fp32

Sharding: data-parallel over images. Each of the 8 cores computes a
[2048, 4096] slice of the output; the small expert table is replicated.

The kernel is HBM-store-bound: the 32 MiB fp32 output per core dwarfs
the inputs, and the 16 SDMA engines stream it at their aggregate line
rate (~420 GB/s). Inputs are shipped in plain fp16 (measured rel err
~3e-4 vs the 2e-2 gate; PSUM accumulates fp32), making the input load
a ~1 MiB footnote. The schedule keeps the store stream saturated from
the earliest point the framework preamble (~9 us) allows:

  - expert table E [64, 4096] is split-packed as [128, 2048]: columns
    0:2048 on partitions 0-63, columns 2048:4096 on partitions 64-127,
    so its load uses all 16 SBUF AXI ports (a [64, x] load would hit
    only the 8 even ports via the partition->port swizzle).
  - gates^T is duplicated onto both partition halves so every (image
    tile, d chunk) pair has lhsT and rhs on the same partition base;
    matmuls run as K=64 tiles at PE quadrant (0,0) or (64,0).
  - input column layout [gt0 | E | gt1 | gt rest] with loads split so
    the first matmul waits only for ~160 KiB.
  - the PE runs cold (1.2 GHz: the HAM activity window never fills
    with independent singly-issued matmuls), so a matmul issues every
    ~0.43 us and a [128,512] fp32 evacuation copy costs ~0.7 us on
    DVE or ACT. A monolithic 2 MiB store needs all 8 of its tile's
    copies (~4.1 us latency) - too late while the head runway drains.
    Hence tiles 0-3 store in 512/1024-column chunks as copies land
    (an ~8 MiB runway), tiles 4-14 in one 2 MiB store each (small
    stores measured ~12% slower ring throughput), and the last tile
    splits into 4 stores to shrink the exposed drain tail.
  - copies alternate DVE/ACT (either alone would serialize); ALL DMA
    dispatches stay on the sync ring, which is idle after the loads
    (~0.6 us per DMA_DIRECT2D dispatch, and dispatches on the scalar
    ring would compete with ACT's copies).
"""

import numpy as np

NUM_EXPERTS = 64
NUM_IMAGES = 16384
D_MODEL = 4096
N_CORES = 8
ROWS = NUM_IMAGES // N_CORES  # 2048 images per core

IMG_TILE = 128          # images per matmul output tile (PSUM partition dim)
N_TILE = 512            # fp32 PSUM bank = 512 floats
HALF = D_MODEL // 2     # E split-pack boundary
OUT_BUFS = 5            # SBUF output staging buffers (bounds DMA in-flight)
RUNWAY_TILES = 3        # leading tiles stored in chunks while the PE ramps
                        # (chunked stores run ~8% below the big-store ring
                        # rate, so the runway is the minimum that still
                        # covers the cold-PE production latency of the
                        # first monolithic tile with ~5 us of margin)

E0 = IMG_TILE           # input column base of the packed E slab
G1 = IMG_TILE + HALF    # input column base of gates tile 1
GR = G1 + IMG_TILE      # input column base of gates tiles 2-15

_CACHE = {}


def _build_module():
    import concourse.bacc as bacc
    import concourse.mybir as mybir
    import concourse.tile as tile

    # Bacc (not bare Bass): its compile() pipeline runs
    # move_matmul_waits_to_ldweights + generate_event_semaphores, which
    # legalize multi-sem-wait instructions (the ISA allows one sync wait
    # per instruction; walrus rejects more).
    nc = bacc.Bacc("TRN2")
    f16 = mybir.dt.float16
    f32 = mybir.dt.float32

    n_img_tiles = ROWS // IMG_TILE          # 16

    with tile.TileContext(nc) as tc:
        with tc.tile_pool(name="dram", bufs=1, space="DRAM") as dram:
            # One packed input per core, column layout [128, 4096]:
            #   cols 0:128      gates^T tile 0, duplicated on both halves
            #   cols 128:2176   E split-packed (d<2048 low / d>=2048 high)
            #   cols 2176:2304  gates^T tile 1, duplicated
            #   cols 2304:4096  gates^T tiles 2-15, duplicated
            allin = dram.tile([128, ROWS + HALF], f16,
                              kind="ExternalInput", name="allin",
                              uniquify=False)
            out = dram.tile([ROWS, D_MODEL], f32, kind="ExternalOutput",
                            name="out", uniquify=False)
            # out[t*128 + p, d] viewed as [p, t, d]: one DMA per image tile
            # covers 128 DRAM rows (16 KiB contiguous each) from one SBUF
            # tile spanning all 128 partitions.
            out_v = out.rearrange("(t p) d -> p t d", p=IMG_TILE)

            with tc.tile_pool(name="const", bufs=1) as cpool, \
                 tc.tile_pool(name="outp", bufs=OUT_BUFS) as outp, \
                 tc.tile_pool(name="psum", bufs=8, space="PSUM") as pspool:
                in_sb = cpool.tile([128, ROWS + HALF], f16, name="in_sb")
                # Loads in dependency order: what tile-0 chunks 0/4 need
                # (gt0 + first E chunk, ONE dma so its completion
                # semaphore — the first matmul's gate — fires earliest),
                # the rest of E, gt1, then the remaining gates.
                nc.sync.dma_start(out=in_sb[:, :E0 + N_TILE],
                                  in_=allin[:, :E0 + N_TILE])
                nc.sync.dma_start(out=in_sb[:, E0 + N_TILE:G1],
                                  in_=allin[:, E0 + N_TILE:G1])
                nc.sync.dma_start(out=in_sb[:, G1:GR], in_=allin[:, G1:GR])
                nc.sync.dma_start(out=in_sb[:, GR:], in_=allin[:, GR:])

                # PE pre-warm on zeros: keeps the tensor engine busy while
                # the leading loads land, so the first real matmuls issue
                # back-to-back instead of paying cold-dispatch latency.
                warm = cpool.tile([64, IMG_TILE + N_TILE], f16, name="warm")
                nc.vector.memset(warm[:], 0)
                for _ in range(6):
                    ps = pspool.tile([128, N_TILE], f32, name="ps")
                    nc.tensor.matmul(ps[:], warm[:, :IMG_TILE],
                                     warm[:, IMG_TILE:], start=True,
                                     stop=True)

                # Chunk order: consecutive chunks alternate PE quadrant
                # (h0/h64), which the tensor engine overlaps (~2x issue
                # rate); for tile 0, chunks 0 and 4 also only need the
                # leading two DMAs, so the store stream starts while the
                # rest of E is still in flight.
                first_order = [0, 4, 1, 5, 2, 6, 3, 7]
                for it in range(n_img_tiles):
                    ot = outp.tile([128, 1, D_MODEL], f32, name="ot")
                    if it == 0:
                        gcol = 0
                    elif it == 1:
                        gcol = G1
                    else:
                        gcol = GR + (it - 2) * IMG_TILE
                    for pos, q in enumerate(first_order):
                        d0 = q * N_TILE
                        base = 0 if d0 < HALF else 64
                        ecol = E0 + d0 % HALF
                        lhsT = in_sb[base:base + 64, gcol:gcol + IMG_TILE]
                        rhs = in_sb[base:base + 64, ecol:ecol + N_TILE]
                        ps = pspool.tile([128, N_TILE], f32, name="ps")
                        nc.tensor.matmul(ps[:], lhsT, rhs,
                                         start=True, stop=True)
                        # Evacuate PSUM; split the copy load between DVE
                        # and ACT by issue position so adjacent chunks'
                        # copies run concurrently.
                        dst = ot[:, 0, d0:d0 + N_TILE]
                        if pos % 2 == 0:
                            nc.vector.tensor_scalar_mul(dst, ps[:], 1.0)
                        else:
                            nc.scalar.mul(dst, ps[:], 1.0)
                        if it == 0:
                            nc.sync.dma_start(
                                out=out_v[:, it, d0:d0 + N_TILE], in_=dst)
                        elif it < RUNWAY_TILES and q % 2 == 1:
                            ds = slice(d0 - N_TILE, d0 + N_TILE)
                            nc.sync.dma_start(out=out_v[:, it, ds],
                                              in_=ot[:, 0, ds])
                    if it == n_img_tiles - 1:
                        # Smaller final DMAs shrink the exposed tail when
                        # one DMA port drains slowly under HBM contention.
                        for h in range(4):
                            cs = slice(h * D_MODEL // 4,
                                       (h + 1) * D_MODEL // 4)
                            nc.sync.dma_start(out=out_v[:, it, cs],
                                              in_=ot[:, 0, cs])
                    elif it >= RUNWAY_TILES:
                        # One 2 MiB DMA per image tile — 1 MiB stores
                        # measured ~12% slower ring throughput.
                        nc.sync.dma_start(out=out_v[:, it:it + 1, :],
                                          in_=ot[:])
    nc.compile()
    return nc


def _get_nc():
    if "nc" not in _CACHE:
        _CACHE["nc"] = _build_module()
    return _CACHE["nc"]


def _make_in_maps(expert_outputs, gates):
    g16 = np.asarray(gates, dtype=np.float16)
    e16 = np.asarray(expert_outputs, dtype=np.float16)
    # E split-pack: [E[:, :2048] ; E[:, 2048:]] -> [128, 2048]
    eslab = np.concatenate([e16[:, :HALF], e16[:, HALF:]], axis=0)

    in_maps = []
    for c in range(N_CORES):
        gt = g16[c * ROWS:(c + 1) * ROWS].T          # [64, 2048]
        gtd = np.concatenate([gt, gt], axis=0)       # [128, 2048] duplicated
        allin = np.ascontiguousarray(
            np.concatenate([gtd[:, :IMG_TILE], eslab,
                            gtd[:, IMG_TILE:]], axis=1))
        in_maps.append({"allin": allin})
    return in_maps


def kernel(expert_outputs: np.ndarray, gates: np.ndarray) -> np.ndarray:
    from concourse.bass_utils import run_bass_kernel_spmd

    nc = _get_nc()
    in_maps = _make_in_maps(expert_outputs, gates)
    res = run_bass_kernel_spmd(nc, in_maps, core_ids=list(range(N_CORES)))
    return np.concatenate([r["out"] for r in res.results], axis=0)
